# revision 13
# baseline (speedup 1.0000x reference)
"""KWinners2d top-k masking kernel for Trainium2 (8 NeuronCores, batch-parallel).

Algorithm (per sample, n = 256*32*32 = 262144, k = 26214):
  boosted y = x * boost[c];  T = k-th largest of y;  out = x * (y >= T).

Exact k-th largest selection on device, per sample:
  1. y = boost_c * x          (ACT, per-partition scale, exact f32 mult)
  2. c0 ~= #{y >= u0}         (ACT Sign + accumulator; +-1 error harmless)
     u0 = build-time quantile of the boosted mixture at tail prob k/n.
  3. u1 = u0 + (c0-(k-300))/(n*pdf)  so that c(u1) ~= k-300 (sub-sample-exact
     Newton step using the true mixture density).  u2 = u1 - 700/(n*pdf).
  4. exact c1 = #{y >= u1} and band count B = #{u2 <= y < u1}  (fused DVE
     tensor_scalar / scalar_tensor_tensor passes with accumulators)
  5. zz = y where in band else -1e30, plus P = 16*(k-c1) - 31 - B pad slots
     of -1e28 (valid, below band).  GPSIMD kth_largest with quantile 15/16
     then computes k_adj = (B+P-1)//16 = r-2 and returns desc[r-1] = exact
     global k-th largest T (r = k - c1 = rank of T within the band).
  6. out = (y >= T) * x       (fused DVE pass)

The pipeline is exact: every count uses exact f32 compares, the band is
guaranteed (prob < 1e-6 otherwise, checked host-side via the stats output
with a numpy fallback per offending sample) to contain rank k with
r in [2,508] so the GPSIMD heap (cap 510) suffices.
"""

import math
from contextlib import ExitStack

import numpy as np

B_FULL = 128
N_CORES = 8
BS = B_FULL // N_CORES          # samples per core
C = 256
HW = 1024                       # 32*32
N = C * HW                      # per-sample elements
K = int(round(N * 0.1))         # 26214
NPAD = 64                       # pad columns in zz
NPL = 2 * HW + NPAD             # kth_largest n_per_lane = 2112
TARGET_GAP = 300.0              # c(u1) target = K - TARGET_GAP
BAND_RANKS = 700.0              # target band width in ranks
VALID_PAD = -1.0e28             # > -1e29  -> counted valid by kth_largest
INVALID = -1.0e30               # < -1e29  -> ignored by kth_largest

_CACHE: dict[bytes, tuple] = {}
TRACE = False          # set True to capture an NTFF profile in LAST_RESULTS
LAST_RESULTS = None


def _mixture_consts(boost: np.ndarray):
    """u0 with P(|mixture| tail >= u0) = K/N, and pdf at u0, for the
    boosted mixture  y ~ (1/C) sum_c N(0, boost_c^2)."""
    b = boost.astype(np.float64)
    target = K / N

    def tail(u):  # P(Y >= u)
        return float(np.mean(0.5 * np.vectorize(math.erfc)(u / (b * math.sqrt(2.0)))))

    lo, hi = 0.0, 20.0
    for _ in range(80):
        mid = 0.5 * (lo + hi)
        if tail(mid) > target:
            lo = mid
        else:
            hi = mid
    u0 = 0.5 * (lo + hi)
    pdf = float(
        np.mean(np.exp(-0.5 * (u0 / b) ** 2) / (b * math.sqrt(2.0 * math.pi)))
    )
    return u0, pdf


def _build(boost: np.ndarray):
    import concourse.bass as bass
    import concourse.mybir as mybir
    from concourse.tile import TileContext

    fp = mybir.dt.float32
    Alu = mybir.AluOpType
    Act = mybir.ActivationFunctionType

    u0, pdf = _mixture_consts(boost)
    inv = 1.0 / (N * pdf)               # value-units per rank
    slope = inv / 2.0
    icept = u0 + (N / 2.0 - K + TARGET_GAP) * inv
    c2 = BAND_RANKS * inv               # u2 = u1 - c2

    import concourse.bacc as bacc
    nc = bacc.Bacc("TRN2", target_bir_lowering=False, debug=False,
                   num_devices=N_CORES)

    x_d = nc.dram_tensor("x", [BS, C, HW], fp, kind="ExternalInput").ap()
    boost_d = nc.dram_tensor("boost", [C, 1], fp, kind="ExternalInput").ap()
    iota_d = nc.dram_tensor("iota", [128, NPAD], fp, kind="ExternalInput").ap()
    out_d = nc.dram_tensor("out", [BS, C, HW], fp, kind="ExternalOutput").ap()
    st_d = nc.dram_tensor("stats", [BS, 8], fp, kind="ExternalOutput").ap()

    from concourse import library_config

    es = ExitStack()
    with TileContext(nc) as tc, es:
        nc.gpsimd.load_library(library_config.attn)
        cpool = es.enter_context(tc.tile_pool(name="const", bufs=1))
        xpool = es.enter_context(tc.tile_pool(name="x", bufs=2))
        ypool = es.enter_context(tc.tile_pool(name="y", bufs=2))
        tpool = es.enter_context(tc.tile_pool(name="t", bufs=2))
        opool = es.enter_context(tc.tile_pool(name="o", bufs=2))
        zpool = es.enter_context(tc.tile_pool(name="z", bufs=2))
        spool = es.enter_context(tc.tile_pool(name="s", bufs=3))
        ppool = es.enter_context(tc.tile_pool(name="ps", bufs=1, space="PSUM"))

        boost_t = cpool.tile([128, 2], fp, tag="boost")
        nc.sync.dma_start(boost_t[:, 0:1], boost_d[0:128, :])
        nc.sync.dma_start(boost_t[:, 1:2], boost_d[128:256, :])
        iota_t = cpool.tile([128, NPAD], fp, tag="iota")
        nc.sync.dma_start(iota_t, iota_d)
        padval = cpool.tile([128, NPAD], fp, tag="padval")
        nc.vector.memset(padval, VALID_PAD)
        onesT = cpool.tile([128, 1], fp, tag="onesT")   # lhsT for col sums
        nc.vector.memset(onesT, 1.0)
        ones1 = cpool.tile([1, 128], fp, tag="ones1")   # lhsT for broadcast
        nc.vector.memset(ones1, 1.0)
        scr = cpool.tile([128, HW], fp, tag="scr")      # sign-output scratch
        negu0 = cpool.tile([128, 1], fp, tag="negu0")
        nc.vector.memset(negu0, -u0)

        for s in range(BS):
            xa = xpool.tile([128, HW], fp, tag="xa")
            xb = xpool.tile([128, HW], fp, tag="xb")
            nc.sync.dma_start(xa, x_d[s, 0:128, :])
            nc.sync.dma_start(xb, x_d[s, 128:256, :])

            ya = ypool.tile([128, HW], fp, tag="ya")
            yb = ypool.tile([128, HW], fp, tag="yb")
            nc.scalar.mul(ya, xa, boost_t[:, 0:1])
            nc.scalar.mul(yb, xb, boost_t[:, 1:2])

            # --- coarse count via sign-sum at u0 ---------------------------
            sgn = spool.tile([128, 2], fp, tag="sgn")
            nc.scalar.activation(scr, ya, Act.Sign, bias=negu0[:, 0:1],
                                 accum_out=sgn[:, 0:1])
            nc.scalar.activation(scr, yb, Act.Sign, bias=negu0[:, 0:1],
                                 accum_out=sgn[:, 1:2])
            psS = ppool.tile([1, 1], fp, tag="psS")
            nc.tensor.matmul(psS, onesT, sgn[:, 0:1], start=True, stop=False)
            nc.tensor.matmul(psS, onesT, sgn[:, 1:2], start=False, stop=True)

            # u1 = slope*S + icept ; u2 = u1 - c2   (packed [1,2])
            u12s = spool.tile([1, 2], fp, tag="u12s")
            nc.vector.tensor_scalar(u12s[0:1, 0:1], psS, slope, icept,
                                    op0=Alu.mult, op1=Alu.add)
            nc.vector.tensor_scalar(u12s[0:1, 1:2], u12s[0:1, 0:1], -c2, None,
                                    op0=Alu.add)
            psU = ppool.tile([128, 2], fp, tag="psU")
            nc.tensor.matmul(psU, ones1, u12s, start=True, stop=True)
            u12 = spool.tile([128, 2], fp, tag="u12")
            nc.vector.tensor_copy(u12, psU)

            # --- exact c1 and band count B ---------------------------------
            ta = tpool.tile([128, HW], fp, tag="ta")
            tb = tpool.tile([128, HW], fp, tag="tb")
            fa = tpool.tile([128, HW], mybir.dt.uint8, tag="fa")
            fb = tpool.tile([128, HW], mybir.dt.uint8, tag="fb")
            acc = spool.tile([128, 4], fp, tag="acc")
            nc.vector.tensor_scalar(ta, ya, u12[:, 0:1], None, op0=Alu.is_ge,
                                    op1=Alu.add, accum_out=acc[:, 0:1])
            nc.vector.tensor_scalar(tb, yb, u12[:, 0:1], None, op0=Alu.is_ge,
                                    op1=Alu.add, accum_out=acc[:, 1:2])
            nc.vector.scalar_tensor_tensor(fa, ya, u12[:, 1:2], ta,
                                           op0=Alu.is_ge, op1=Alu.subtract,
                                           accum_out=acc[:, 2:3])
            nc.vector.scalar_tensor_tensor(fb, yb, u12[:, 1:2], tb,
                                           op0=Alu.is_ge, op1=Alu.subtract,
                                           accum_out=acc[:, 3:4])
            psA = ppool.tile([1, 2], fp, tag="psA")     # [c1, B]
            nc.tensor.matmul(psA, onesT, acc[:, 0:4:2], start=True, stop=False)
            nc.tensor.matmul(psA, onesT, acc[:, 1:4:2], start=False, stop=True)

            # r = clamp(K - c1, 2, 508) ; P = 16r - B - 31 (>= 0)
            rP = spool.tile([1, 2], fp, tag="rP")
            nc.vector.tensor_scalar(rP[0:1, 0:1], psA[0:1, 0:1], -1.0, float(K),
                                    op0=Alu.mult, op1=Alu.add)
            nc.vector.tensor_scalar(rP[0:1, 0:1], rP[0:1, 0:1], 2.0, 508.0,
                                    op0=Alu.max, op1=Alu.min)
            nc.vector.scalar_tensor_tensor(rP[0:1, 1:2], rP[0:1, 0:1], 16.0,
                                           psA[0:1, 1:2],
                                           op0=Alu.mult, op1=Alu.subtract)
            nc.vector.tensor_scalar(rP[0:1, 1:2], rP[0:1, 1:2], -31.0, 0.0,
                                    op0=Alu.add, op1=Alu.max)
            psP = ppool.tile([128, 1], fp, tag="psP")
            nc.tensor.matmul(psP, ones1, rP[0:1, 1:2], start=True, stop=True)

            # --- zz: band values + P valid pads ---------------------------
            zz = zpool.tile([128, NPL], fp, tag="zz")
            nc.gpsimd.memset(zz, INVALID)
            nc.vector.copy_predicated(zz[:, 0:HW], fa, ya)
            nc.vector.copy_predicated(zz[:, HW:2 * HW], fb, yb)
            pm = spool.tile([128, NPAD], mybir.dt.uint8, tag="pm")
            nc.vector.tensor_scalar(pm, iota_t, psP, None, op0=Alu.is_lt)
            nc.vector.copy_predicated(zz[:, 2 * HW:], pm, padval)

            kout = spool.tile([1, 2], fp, tag="kout")
            nc.gpsimd.kth_largest(kout, zz, n_per_lane=NPL, k=510,
                                  quantile=1.0 - 1.0 / 16.0)

            psT = ppool.tile([128, 1], fp, tag="psT")
            nc.tensor.matmul(psT, ones1, kout[0:1, 1:2], start=True, stop=True)
            Tb = spool.tile([128, 1], fp, tag="Tb")
            nc.vector.tensor_copy(Tb, psT)

            # --- final mask ------------------------------------------------
            oa = opool.tile([128, HW], fp, tag="oa")
            ob = opool.tile([128, HW], fp, tag="ob")
            nc.vector.scalar_tensor_tensor(oa, ya, Tb, xa,
                                           op0=Alu.is_ge, op1=Alu.mult)
            nc.vector.scalar_tensor_tensor(ob, yb, Tb, xb,
                                           op0=Alu.is_ge, op1=Alu.mult)
            nc.sync.dma_start(out_d[s, 0:128, :], oa)
            nc.sync.dma_start(out_d[s, 128:256, :], ob)

            stt = spool.tile([1, 8], fp, tag="stt")
            nc.vector.tensor_copy(stt[0:1, 0:2], psA)        # c1, B
            nc.vector.tensor_copy(stt[0:1, 2:4], rP)         # r, P
            nc.vector.tensor_copy(stt[0:1, 4:6], kout)       # lerp, T
            nc.vector.memset(stt[0:1, 6:8], 0.0)
            nc.sync.dma_start(st_d[s:s + 1, :], stt)

    nc.compile()
    return nc


def _get_program(boost: np.ndarray):
    key = boost.tobytes()
    if key not in _CACHE:
        _CACHE[key] = _build(boost)
    return _CACHE[key]


def _boost_from_duty(dutyCycle: np.ndarray) -> np.ndarray:
    # computed with jax-on-CPU to bit-match the reference's jnp.exp
    import jax
    import jax.numpy as jnp
    target_density = float(K) / float(N)
    cpu = jax.devices("cpu")[0]
    with jax.default_device(cpu):
        d = jax.device_put(np.asarray(dutyCycle), cpu)
        boost = jnp.exp((target_density - d) * 1.0)
    return np.asarray(boost, dtype=np.float32).reshape(C)


def kernel(x: np.ndarray, dutyCycle: np.ndarray) -> np.ndarray:
    from concourse import bass_utils

    x = np.ascontiguousarray(x, dtype=np.float32)
    boost = _boost_from_duty(dutyCycle)
    nc = _get_program(boost)

    xr = x.reshape(N_CORES, BS, C, HW)
    boost_in = boost.reshape(C, 1)
    iota_in = (np.arange(128 * NPAD, dtype=np.float32)
               .reshape(128, NPAD))
    in_maps = [{"x": xr[c], "boost": boost_in, "iota": iota_in}
               for c in range(N_CORES)]
    try:
        res = bass_utils.run_bass_kernel_spmd(nc, in_maps,
                                              core_ids=list(range(N_CORES)),
                                              trace=TRACE)
    except ModuleNotFoundError:
        # no NTFF profiling hook in this container — run untraced
        res = bass_utils.run_bass_kernel_spmd(nc, in_maps,
                                              core_ids=list(range(N_CORES)))
    global LAST_RESULTS
    LAST_RESULTS = res
    out = np.concatenate([res.results[c]["out"][None] for c in range(N_CORES)])
    out = out.reshape(B_FULL, C, 32, 32)
    stats = np.concatenate([res.results[c]["stats"][None]
                            for c in range(N_CORES)]).reshape(B_FULL, 8)

    # host-side validity guard (prob ~1e-6); numpy fallback per bad sample
    c1, B = stats[:, 0], stats[:, 1]
    r = K - c1
    P = 16.0 * r - B - 31.0
    bad = (r < 2) | (r > 508) | (r > B) | (P < 0) | (P > 8191)
    if bad.any():
        for s in np.nonzero(bad)[0]:
            boosted = (x[s].reshape(C, HW) * boost[:, None]).ravel()
            thr = np.partition(boosted, N - K)[N - K]
            out[s] = (x[s].reshape(C, HW)
                      * (boosted.reshape(C, HW) >= thr)).reshape(C, 32, 32)
    return out


# revision 15
# speedup vs baseline: 1.0807x; 1.0807x over previous
"""KWinners2d top-k masking kernel for Trainium2 (8 NeuronCores, batch-parallel).

Algorithm (per sample, n = 256*32*32 = 262144, k = 26214):
  boosted y = x * boost[c];  T = k-th largest of y;  out = x * (y >= T).

Exact k-th largest selection on device, per sample:
  1. y = boost_c * x          (ACT, per-partition scale, exact f32 mult)
  2. c0 ~= #{y >= u0}         (ACT Sign + accumulator; +-1 error harmless)
     u0 = build-time quantile of the boosted mixture at tail prob k/n.
  3. u1 = u0 + (c0-(k-300))/(n*pdf)  so that c(u1) ~= k-300 (sub-sample-exact
     Newton step using the true mixture density).  u2 = u1 - 700/(n*pdf).
  4. exact c1 = #{y >= u1} and band count B = #{u2 <= y < u1}  (fused DVE
     tensor_scalar / scalar_tensor_tensor passes with accumulators)
  5. zz = y where in band else -1e30, plus P = 16*(k-c1) - 31 - B pad slots
     of -1e28 (valid, below band).  GPSIMD kth_largest with quantile 15/16
     then computes k_adj = (B+P-1)//16 = r-2 and returns desc[r-1] = exact
     global k-th largest T (r = k - c1 = rank of T within the band).
  6. out = (y >= T) * x       (fused DVE pass)

The pipeline is exact: every count uses exact f32 compares, the band is
guaranteed (prob < 1e-6 otherwise, checked host-side via the stats output
with a numpy fallback per offending sample) to contain rank k with
r in [2,508] so the GPSIMD heap (cap 510) suffices.
"""

import math
from contextlib import ExitStack

import numpy as np

B_FULL = 128
N_CORES = 8
BS = B_FULL // N_CORES          # samples per core
C = 256
HW = 1024                       # 32*32
N = C * HW                      # per-sample elements
K = int(round(N * 0.1))         # 26214
NPAD = 64                       # pad columns in zz
NPL = 2 * HW + NPAD             # kth_largest n_per_lane = 2112
TARGET_GAP = 300.0              # c(u1) target = K - TARGET_GAP
BAND_RANKS = 700.0              # target band width in ranks
VALID_PAD = -1.0e28             # > -1e29  -> counted valid by kth_largest
INVALID = -1.0e30               # < -1e29  -> ignored by kth_largest

_CACHE: dict[bytes, tuple] = {}
TRACE = False          # set True to capture an NTFF profile in LAST_RESULTS
LAST_RESULTS = None


def _mixture_consts(boost: np.ndarray):
    """u0 with P(|mixture| tail >= u0) = K/N, and pdf at u0, for the
    boosted mixture  y ~ (1/C) sum_c N(0, boost_c^2)."""
    b = boost.astype(np.float64)
    target = K / N

    def tail(u):  # P(Y >= u)
        return float(np.mean(0.5 * np.vectorize(math.erfc)(u / (b * math.sqrt(2.0)))))

    lo, hi = 0.0, 20.0
    for _ in range(80):
        mid = 0.5 * (lo + hi)
        if tail(mid) > target:
            lo = mid
        else:
            hi = mid
    u0 = 0.5 * (lo + hi)
    pdf = float(
        np.mean(np.exp(-0.5 * (u0 / b) ** 2) / (b * math.sqrt(2.0 * math.pi)))
    )
    return u0, pdf


def _build(boost: np.ndarray):
    import concourse.bass as bass
    import concourse.mybir as mybir
    from concourse.tile import TileContext

    fp = mybir.dt.float32
    Alu = mybir.AluOpType
    Act = mybir.ActivationFunctionType

    u0, pdf = _mixture_consts(boost)
    inv = 1.0 / (N * pdf)               # value-units per rank
    slope = inv / 2.0
    icept = u0 + (N / 2.0 - K + TARGET_GAP) * inv
    c2 = BAND_RANKS * inv               # u2 = u1 - c2

    import concourse.bacc as bacc
    nc = bacc.Bacc("TRN2", target_bir_lowering=False, debug=False,
                   num_devices=N_CORES)

    x_d = nc.dram_tensor("x", [BS, C, HW], fp, kind="ExternalInput").ap()
    boost_d = nc.dram_tensor("boost", [C, 1], fp, kind="ExternalInput").ap()
    iota_d = nc.dram_tensor("iota", [128, NPAD], fp, kind="ExternalInput").ap()
    out_d = nc.dram_tensor("out", [BS, C, HW], fp, kind="ExternalOutput").ap()
    st_d = nc.dram_tensor("stats", [BS, 8], fp, kind="ExternalOutput").ap()

    from concourse import library_config

    es = ExitStack()
    with TileContext(nc) as tc, es:
        nc.gpsimd.load_library(library_config.attn)
        cpool = es.enter_context(tc.tile_pool(name="const", bufs=1))
        xpool = es.enter_context(tc.tile_pool(name="x", bufs=2))
        ypool = es.enter_context(tc.tile_pool(name="y", bufs=2))
        tpool = es.enter_context(tc.tile_pool(name="t", bufs=2))
        opool = es.enter_context(tc.tile_pool(name="o", bufs=2))
        zpool = es.enter_context(tc.tile_pool(name="z", bufs=2))
        spool = es.enter_context(tc.tile_pool(name="s", bufs=3))
        ppool = es.enter_context(tc.tile_pool(name="ps", bufs=1, space="PSUM"))

        boost_t = cpool.tile([128, 2], fp, tag="boost")
        nc.sync.dma_start(boost_t[:, 0:1], boost_d[0:128, :])
        nc.sync.dma_start(boost_t[:, 1:2], boost_d[128:256, :])
        iota_t = cpool.tile([128, NPAD], fp, tag="iota")
        nc.sync.dma_start(iota_t, iota_d)
        padval = cpool.tile([128, NPAD], fp, tag="padval")
        nc.vector.memset(padval, VALID_PAD)
        onesT = cpool.tile([128, 1], fp, tag="onesT")   # lhsT for col sums
        nc.vector.memset(onesT, 1.0)
        ones1 = cpool.tile([1, 128], fp, tag="ones1")   # lhsT for broadcast
        nc.vector.memset(ones1, 1.0)
        scr = cpool.tile([128, HW], fp, tag="scr")      # sign-output scratch
        negu0 = cpool.tile([128, 1], fp, tag="negu0")
        nc.vector.memset(negu0, -u0)

        for s in range(BS):
            xa = xpool.tile([128, HW], fp, tag="xa")
            xb = xpool.tile([128, HW], fp, tag="xb")
            nc.sync.dma_start(xa, x_d[s, 0:128, :])
            nc.sync.dma_start(xb, x_d[s, 128:256, :])

            ya = ypool.tile([128, HW], fp, tag="ya")
            yb = ypool.tile([128, HW], fp, tag="yb")
            nc.scalar.mul(ya, xa, boost_t[:, 0:1])
            nc.scalar.mul(yb, xb, boost_t[:, 1:2])

            # --- coarse count via sign-sum at u0 ---------------------------
            sgn = spool.tile([128, 2], fp, tag="sgn")
            nc.scalar.activation(scr, ya, Act.Sign, bias=negu0[:, 0:1],
                                 accum_out=sgn[:, 0:1])
            nc.scalar.activation(scr, yb, Act.Sign, bias=negu0[:, 0:1],
                                 accum_out=sgn[:, 1:2])
            psS = ppool.tile([1, 1], fp, tag="psS")
            nc.tensor.matmul(psS, onesT, sgn[:, 0:1], start=True, stop=False)
            nc.tensor.matmul(psS, onesT, sgn[:, 1:2], start=False, stop=True)

            # u1 = slope*S + icept ; u2 = u1 - c2   (packed [1,2])
            u12s = spool.tile([1, 2], fp, tag="u12s")
            nc.vector.tensor_scalar(u12s[0:1, 0:1], psS, slope, icept,
                                    op0=Alu.mult, op1=Alu.add)
            nc.vector.tensor_scalar(u12s[0:1, 1:2], u12s[0:1, 0:1], -c2, None,
                                    op0=Alu.add)
            psU = ppool.tile([128, 2], fp, tag="psU")
            nc.tensor.matmul(psU, ones1, u12s, start=True, stop=True)
            u12 = spool.tile([128, 2], fp, tag="u12")
            nc.vector.tensor_copy(u12, psU)

            # --- exact c1 and band count B ---------------------------------
            ta = tpool.tile([128, HW], fp, tag="ta")
            tb = tpool.tile([128, HW], fp, tag="tb")
            fa = tpool.tile([128, HW], mybir.dt.uint8, tag="fa")
            fb = tpool.tile([128, HW], mybir.dt.uint8, tag="fb")
            acc = spool.tile([128, 4], fp, tag="acc")
            nc.vector.tensor_scalar(ta, ya, u12[:, 0:1], None, op0=Alu.is_ge,
                                    op1=Alu.add, accum_out=acc[:, 0:1])
            nc.vector.tensor_scalar(tb, yb, u12[:, 0:1], None, op0=Alu.is_ge,
                                    op1=Alu.add, accum_out=acc[:, 1:2])
            nc.vector.scalar_tensor_tensor(fa, ya, u12[:, 1:2], ta,
                                           op0=Alu.is_ge, op1=Alu.subtract,
                                           accum_out=acc[:, 2:3])
            nc.vector.scalar_tensor_tensor(fb, yb, u12[:, 1:2], tb,
                                           op0=Alu.is_ge, op1=Alu.subtract,
                                           accum_out=acc[:, 3:4])
            psA = ppool.tile([1, 2], fp, tag="psA")     # [c1, B]
            nc.tensor.matmul(psA, onesT, acc[:, 0:4:2], start=True, stop=False)
            nc.tensor.matmul(psA, onesT, acc[:, 1:4:2], start=False, stop=True)

            # r = clamp(K - c1, 2, 508) ; P = 16r - B - 31 (>= 0)
            rP = spool.tile([1, 2], fp, tag="rP")
            nc.vector.tensor_scalar(rP[0:1, 0:1], psA[0:1, 0:1], -1.0, float(K),
                                    op0=Alu.mult, op1=Alu.add)
            nc.vector.tensor_scalar(rP[0:1, 0:1], rP[0:1, 0:1], 2.0, 508.0,
                                    op0=Alu.max, op1=Alu.min)
            nc.vector.scalar_tensor_tensor(rP[0:1, 1:2], rP[0:1, 0:1], 16.0,
                                           psA[0:1, 1:2],
                                           op0=Alu.mult, op1=Alu.subtract)
            nc.vector.tensor_scalar(rP[0:1, 1:2], rP[0:1, 1:2], -31.0, 0.0,
                                    op0=Alu.add, op1=Alu.max)
            psP = ppool.tile([128, 1], fp, tag="psP")
            nc.tensor.matmul(psP, ones1, rP[0:1, 1:2], start=True, stop=True)

            # --- zz: band values + P valid pads ---------------------------
            zz = zpool.tile([128, NPL], fp, tag="zz")
            nc.gpsimd.memset(zz, INVALID)
            nc.vector.copy_predicated(zz[:, 0:HW], fa, ya)
            nc.vector.copy_predicated(zz[:, HW:2 * HW], fb, yb)
            pm = spool.tile([128, NPAD], mybir.dt.uint8, tag="pm")
            nc.vector.tensor_scalar(pm, iota_t, psP, None, op0=Alu.is_lt)
            nc.vector.copy_predicated(zz[:, 2 * HW:], pm, padval)

            kout = spool.tile([1, 2], fp, tag="kout")
            nc.gpsimd.kth_largest(kout, zz, n_per_lane=NPL, k=510,
                                  quantile=1.0 - 1.0 / 16.0)

            psT = ppool.tile([128, 1], fp, tag="psT")
            nc.tensor.matmul(psT, ones1, kout[0:1, 1:2], start=True, stop=True)
            Tb = spool.tile([128, 1], fp, tag="Tb")
            nc.vector.tensor_copy(Tb, psT)

            # --- final mask ------------------------------------------------
            oa = opool.tile([128, HW], fp, tag="oa")
            ob = opool.tile([128, HW], fp, tag="ob")
            nc.vector.scalar_tensor_tensor(oa, ya, Tb, xa,
                                           op0=Alu.is_ge, op1=Alu.mult)
            nc.vector.scalar_tensor_tensor(ob, yb, Tb, xb,
                                           op0=Alu.is_ge, op1=Alu.mult)
            nc.sync.dma_start(out_d[s, 0:128, :], oa)
            nc.sync.dma_start(out_d[s, 128:256, :], ob)

            nc.sync.dma_start(st_d[s:s + 1, 2:4], rP)        # r, P
            nc.sync.dma_start(st_d[s:s + 1, 4:6], kout)      # lerp, T

    nc.compile()
    return nc


def _get_program(boost: np.ndarray):
    key = boost.tobytes()
    if key not in _CACHE:
        _CACHE[key] = _build(boost)
    return _CACHE[key]


def _boost_from_duty(dutyCycle: np.ndarray) -> np.ndarray:
    # computed with jax-on-CPU to bit-match the reference's jnp.exp
    import jax
    import jax.numpy as jnp
    target_density = float(K) / float(N)
    cpu = jax.devices("cpu")[0]
    with jax.default_device(cpu):
        d = jax.device_put(np.asarray(dutyCycle), cpu)
        boost = jnp.exp((target_density - d) * 1.0)
    return np.asarray(boost, dtype=np.float32).reshape(C)


def kernel(x: np.ndarray, dutyCycle: np.ndarray) -> np.ndarray:
    from concourse import bass_utils

    x = np.ascontiguousarray(x, dtype=np.float32)
    boost = _boost_from_duty(dutyCycle)
    nc = _get_program(boost)

    xr = x.reshape(N_CORES, BS, C, HW)
    boost_in = boost.reshape(C, 1)
    iota_in = (np.arange(128 * NPAD, dtype=np.float32)
               .reshape(128, NPAD))
    in_maps = [{"x": xr[c], "boost": boost_in, "iota": iota_in}
               for c in range(N_CORES)]
    try:
        res = bass_utils.run_bass_kernel_spmd(nc, in_maps,
                                              core_ids=list(range(N_CORES)),
                                              trace=TRACE)
    except ModuleNotFoundError:
        # no NTFF profiling hook in this container — run untraced
        res = bass_utils.run_bass_kernel_spmd(nc, in_maps,
                                              core_ids=list(range(N_CORES)))
    global LAST_RESULTS
    LAST_RESULTS = res
    out = np.concatenate([res.results[c]["out"][None] for c in range(N_CORES)])
    out = out.reshape(B_FULL, C, 32, 32)
    stats = np.concatenate([res.results[c]["stats"][None]
                            for c in range(N_CORES)]).reshape(B_FULL, 8)

    # host-side validity guard (prob ~1e-6); numpy fallback per bad sample.
    # r,P were clamped on device; clamp-bound values mark invalid samples.
    r, P = stats[:, 2], stats[:, 3]
    B = 16.0 * r - 31.0 - P
    bad = (r <= 2) | (r >= 508) | (P <= 0) | (P > 8191) | (r > B)
    if bad.any():
        for s in np.nonzero(bad)[0]:
            boosted = (x[s].reshape(C, HW) * boost[:, None]).ravel()
            thr = np.partition(boosted, N - K)[N - K]
            out[s] = (x[s].reshape(C, HW)
                      * (boosted.reshape(C, HW) >= thr)).reshape(C, 32, 32)
    return out
